# revision 30
# baseline (speedup 1.0000x reference)
"""Trainium2 Bass kernel for nn_Actor (ragged GTrXL-style actor network).

Pure data parallel over 8 NeuronCores: 8 samples per core. The full forward
(one-hot voxel featurization via a hinge-basis decomposition, rel-pos
attention with ragged masks, two GRU gates, LN/MLP, voxel-mean head,
value/action heads) runs on-device in a single NEFF; the host only shards
inputs / reassembles outputs.

Self-contained: hardcodes all shapes; no sibling imports.
"""
import sys

if "/opt/trn_rl_repo" not in sys.path:
    sys.path.insert(0, "/opt/trn_rl_repo")

import numpy as np
import ml_dtypes
BF16 = ml_dtypes.bfloat16

# ---- problem constants ----
B, T_OBS, GF = 64, 101, 7
T = 100
NCELL, NMAT = 1000, 4
VDIM = NCELL * NMAT          # 4000
D, H, HD = 64, 2, 32
GDIM, VFEAT = 16, 48
TAU = 50
MLP_D = 32
NOUT = 16
EPS = 1e-5
NCORES = 8
BC = B // NCORES             # 8 samples / core
NTOK = BC * T                # 800 tokens / core
FT = TAU + T                 # 150
NFTOK = BC * FT              # 1200
INV_SQRT_HD = 1.0 / np.sqrt(HD)

# blob64 column layout
_B64 = {}
_off = 0
for _name, _w in [("wqkv", 192), ("wpos", 64), ("wattn", 64), ("g1w", 384),
                  ("g2w", 384), ("we1", 32), ("we2", 64), ("wa1", 64),
                  ("wa2", 16), ("wval1", 64), ("wval2", 1), ("relposT", 150),
                  ("ba1", 1), ("bval1", 1), ("ba2", 1), ("bval2", 1),
                  ("g1bz", 1), ("g2bz", 1), ("u", 1), ("v", 1), ("wg", 16)]:
    _B64[_name] = (_off, _w)
    _off += _w
NB64 = _off
# blob100 column layout: valid 0:8 | avmask 8:16 | causal 16:166
NB100 = 166
# blobR: ln1g 0:64 | ln1b 64:128 | ln2g 128:192 | ln2b 192:256 | bv 256:304
NBR = 304

_CACHE = {}


def _relpos_table():
    inv = 1.0 / (10000.0 ** (np.arange(0, D, 2, dtype=np.float32) / D))
    pos = np.arange(FT - 1, -1, -1, dtype=np.float32)
    ang = pos[:, None] * inv[None, :]
    return np.concatenate([np.sin(ang), np.cos(ang)], -1).T.copy()  # [64, 150]


def _build():
    import concourse.bass as bass
    import concourse.tile as tile
    from concourse import bacc, mybir
    from concourse.masks import make_identity
    from contextlib import ExitStack

    f32 = mybir.dt.float32
    bf = mybir.dt.bfloat16
    i8 = mybir.dt.int8
    AF = mybir.ActivationFunctionType
    OP = mybir.AluOpType
    AX = mybir.AxisListType

    nc = bacc.Bacc("TRN2", target_bir_lowering=False, debug=False,
                   enable_asserts=True, num_devices=NCORES)

    def din(name, shape, dt=f32):
        return nc.declare_dram_parameter(name, list(shape), dt, isOutput=False)

    # inputs
    vox = din("vox", (NCELL, NTOK), i8)
    gaus = din("gaus", (GF + 1, NTOK), bf)
    st0 = din("st0", (D, BC * TAU), bf)
    aug2 = din("aug2", (2, NTOK), bf)
    sel = din("sel", (1, NTOK), bf)
    blob64 = din("blob64", (D, NB64), bf)
    blob100 = din("blob100", (T, NB100))
    blobr = din("blobr", (1, NBR))
    wvm = din("wvm", (NMAT, NCELL, VFEAT), bf)
    wvox = din("wvox", (D, VDIM), bf)
    bvox = din("bvox", (VDIM,), bf)
    bvr = din("bvr", (1, VFEAT), bf)
    # outputs
    ovox = nc.declare_dram_parameter("ovox", [NTOK, VDIM], f32, isOutput=True)
    oact = nc.declare_dram_parameter("oact", [NOUT, BC], f32, isOutput=True)
    oval = nc.declare_dram_parameter("oval", [1, BC], f32, isOutput=True)

    KT = [128] * 7 + [104]                          # cell tiles
    CH = [(0, 512), (512, 288)]                     # NTOK chunks (<=512)
    CH3 = [(0, 512), (512, 512), (1024, 176)]       # NFTOK chunks

    with tile.TileContext(nc) as tc, ExitStack() as ctx:
        dma = nc.sync.dma_start

        pconst = ctx.enter_context(tc.tile_pool(name="pconst", bufs=1))
        pbig = ctx.enter_context(tc.tile_pool(name="pbig", bufs=1))
        pwarm = ctx.enter_context(tc.tile_pool(name="pwarm", bufs=1, space="PSUM"))
        warm_ps = pwarm.tile([D, D], f32, tag="warm")
        warm_src = [None]

        def warm(n=2, anchor=None):
            if warm_src[0] is None:
                return
            for _ in range(n):
                nc.tensor.matmul(warm_ps[0:64, 0:64], warm_src[0], warm_src[0],
                                 start=True, stop=True)

        # ---------- early bulk input DMAs (longest supply chains) ----------
        vox_tiles = []
        KT0 = [128] * 7 + [104]
        for k in range(8):
            vt_ = pconst.tile([128, NTOK], i8, tag=f"vt{k}")
            dma(out=vt_[0:KT0[k], :], in_=vox[128 * k:128 * k + KT0[k], :])
            vox_tiles.append(vt_)
        wvm_tiles = []
        for m in range(NMAT):
            t_ = pconst.tile([128, 8, VFEAT], bf, tag=f"wm{m}")
            nc.vector.memset(t_[96:128, 7, :], 0.0)
            dma(out=t_[:, 0:7, :],
                in_=wvm[m, 0:896, :].rearrange("(k p) f -> p k f", p=128))
            dma(out=t_[0:104, 7, :], in_=wvm[m, 896:1000, :])
            wvm_tiles.append(t_)

        # ---------- constants / weights into SBUF ----------
        ident = pconst.tile([128, 128], bf, tag="ident")
        make_identity(nc, ident[:])

        b64 = pconst.tile([D, NB64], bf, tag="b64")
        wg_aug = None
        dma(out=b64[:], in_=blob64[:])

        def b64s(name):
            o, w = _B64[name]
            return b64[:, o:o + w]

        b100 = pconst.tile([T, NB100], f32, tag="b100")
        dma(out=b100[:], in_=blob100[:])
        valid_sb = b100[:, 0:BC]
        avmask_sb = b100[:, BC:2 * BC]
        causal_sb = b100[:, 2 * BC:2 * BC + FT]


        br = pconst.tile([1, NBR], f32, tag="br")
        dma(out=br[:], in_=blobr[:])

        gaus_sb = pconst.tile([GF + 1, NTOK], bf, tag="gaus")
        dma(out=gaus_sb[:], in_=gaus[:])
        aug_ones = pconst.tile([1, NTOK], bf, tag="augo")
        dma(out=aug_ones[:], in_=aug2[0, :].unsqueeze(0))
        aug_c0 = pconst.tile([1, NTOK], bf, tag="augc")
        dma(out=aug_c0[:], in_=aug2[1, :].unsqueeze(0))
        sel_sb = pconst.tile([1, NTOK], bf, tag="sel")
        dma(out=sel_sb[:], in_=sel[:])

        warm_src[0] = b64s("wqkv")[:, 0:64]
        wqkv_sb = b64s("wqkv")
        wpos_sb = b64s("wpos")
        wattn_sb = b64s("wattn")
        g1w_sb = b64s("g1w").rearrange("d (i m) -> d i m", i=6)
        g2w_sb = b64s("g2w").rearrange("d (i m) -> d i m", i=6)
        we1_sb = b64s("we1")
        we2_sb = b64s("we2")[0:MLP_D, :]
        wa1_sb = b64s("wa1")
        wa2_sb = b64s("wa2")
        wval1_sb = b64s("wval1")
        wval2_sb = b64s("wval2")
        relpos_sb = b64s("relposT")
        wg_aug = b64s("wg")[0:GF + 1, :]
        ba1_c = b64s("ba1")
        bval1_c = b64s("bval1")
        ba2_c = b64s("ba2")[0:NOUT, :]
        bval2_c = b64s("bval2")[0:1, :]

        u_cs = pconst.tile([D, 1], f32, tag="ucs")
        v_cs = pconst.tile([D, 1], f32, tag="vcs")
        nc.vector.tensor_scalar_mul(u_cs[:], b64s("u"), INV_SQRT_HD)
        nc.vector.tensor_scalar_mul(v_cs[:], b64s("v"), INV_SQRT_HD)
        negbz1 = pconst.tile([D, 1], f32, tag="nbz1")
        negbz2 = pconst.tile([D, 1], f32, tag="nbz2")
        nc.vector.tensor_scalar_mul(negbz1[:], b64s("g1bz"), -1.0)
        nc.vector.tensor_scalar_mul(negbz2[:], b64s("g2bz"), -1.0)

        g1_row = br[:, 0:64]
        b1_row = br[:, 64:128]
        g2_row = br[:, 128:192]
        b2_row = br[:, 192:256]
        ng1_row = pconst.tile([1, D], f32, tag="ng1")
        ng2_row = pconst.tile([1, D], f32, tag="ng2")
        nc.vector.tensor_scalar_mul(ng1_row[:], g1_row, -1.0)
        nc.vector.tensor_scalar_mul(ng2_row[:], g2_row, -1.0)
        ones_row = pconst.tile([1, NFTOK], f32, tag="onesrow")
        nc.vector.memset(ones_row[:], 1.0)
        ones128 = pconst.tile([128, 1], bf, tag="ones128")
        nc.vector.memset(ones128[:], 1.0)
        ones1x64 = pconst.tile([1, D], bf, tag="ones1x64")
        nc.vector.memset(ones1x64[:], 1.0)
        cm2 = pconst.tile([128, 1], f32, tag="cm2")
        nc.vector.memset(cm2[:], -2.0)
        ceps = pconst.tile([1, 1], f32, tag="ceps")
        nc.vector.memset(ceps[:], EPS)

        wvoxb = pconst.tile([D + 1, VDIM], bf, tag="wvoxb")
        dma(out=wvoxb[0:D, :], in_=wvox[:])
        dma(out=wvoxb[D:D + 1, :], in_=bvox[:].unsqueeze(0))

        # ---------- phase V: voxel hinge-basis features ----------
        W0, W1, W2, W3 = wvm_tiles
        C1 = pconst.tile([128, 8, VFEAT], bf, tag="c1")
        C2 = pconst.tile([128, 8, VFEAT], bf, tag="c2")
        C3 = pconst.tile([128, 8, VFEAT], bf, tag="c3")
        tmpC = pconst.tile([128, 8, VFEAT], bf, tag="tmpc")
        nc.vector.tensor_sub(C1[:], W1[:], W0[:])
        nc.vector.scalar_tensor_tensor(tmpC[:], W1[:], -2.0, W2[:], OP.mult, OP.add)
        nc.vector.tensor_add(C2[:], tmpC[:], W0[:])
        nc.vector.scalar_tensor_tensor(C3[:], W2[:], -2.0, W3[:], OP.mult, OP.add)
        nc.vector.tensor_add(C3[:], C3[:], W1[:])
        CB = [C1, C2, C3]

        with tc.tile_pool(name="ps_w0", bufs=1, space="PSUM") as ps_w0:
            ps_sw0 = ps_w0.tile([1, VFEAT], f32)
            for k in range(8):
                nc.tensor.matmul(ps_sw0[:], ones128[0:KT[k], :], W0[0:KT[k], k, :],
                                 start=(k == 0), stop=(k == 7))
            sw0_stage = pconst.tile([1, VFEAT], bf, tag="sw0")
            nc.vector.tensor_copy(sw0_stage[:], ps_sw0[:])
        bvr_sb = pconst.tile([1, VFEAT], bf, tag="bvrs")
        dma(out=bvr_sb[:], in_=bvr[:])

        xT = pbig.tile([D, NTOK], bf, tag="xT")

        with tc.tile_pool(name="pmask", bufs=3) as pmask, \
             tc.tile_pool(name="ps_vf", bufs=1, space="PSUM") as ps_vfp, \
             tc.tile_pool(name="ps_g", bufs=1, space="PSUM") as ps_gp:
            ps_vf = ps_vfp.tile([VFEAT, 2, 512], f32)
            for k in range(8):
                kn = KT[k]
                vtile = vox_tiles[k]
                vf32 = pmask.tile([128, NTOK], bf, tag="m0")
                r1 = pmask.tile([128, NTOK], bf, tag="m1")
                r2 = pmask.tile([128, NTOK], bf, tag="m2")
                warm(2)
                nc.vector.tensor_copy(vf32[0:kn, :], vtile[0:kn, :])
                nc.vector.tensor_scalar(r1[0:kn, :], vf32[0:kn, :], 1.0, 0.0,
                                        OP.subtract, OP.max)
                nc.scalar.activation(r2[0:kn, :], vf32[0:kn, :], AF.Relu,
                                     bias=cm2[0:kn, :])
                for ci, (c0, cn) in enumerate(CH):
                    for bi, mask in enumerate((vf32, r1, r2)):
                        nc.tensor.matmul(ps_vf[:, ci, 0:cn], CB[bi][0:kn, k, :],
                                         mask[0:kn, c0:c0 + cn],
                                         start=(k == 0 and bi == 0), stop=False)
            for ci, (c0, cn) in enumerate(CH):
                nc.tensor.matmul(ps_vf[:, ci, 0:cn], bvr_sb[:],
                                 aug_ones[:, c0:c0 + cn], start=False, stop=False)
                nc.tensor.matmul(ps_vf[:, ci, 0:cn], sw0_stage[:],
                                 aug_c0[:, c0:c0 + cn], start=False, stop=True)
            ps_gt = ps_gp.tile([GDIM, 2, 512], f32)
            vf_stage = pmask.tile([VFEAT, NTOK], bf, tag="vfst")
            for ci, (c0, cn) in enumerate(CH):
                nc.tensor.matmul(ps_gt[:, ci, 0:cn], wg_aug,
                                 gaus_sb[:, c0:c0 + cn], start=True, stop=True)
                nc.scalar.copy(xT[0:GDIM, c0:c0 + cn], ps_gt[:, ci, 0:cn])
                nc.scalar.copy(vf_stage[:, c0:c0 + cn], ps_vf[:, ci, 0:cn])
            for c0, cn in CH:
                dma(out=xT[GDIM:D, c0:c0 + cn], in_=vf_stage[:, c0:c0 + cn])

        # ---------- fullT = [mem | x] per sample ----------
        pA_cm = tc.tile_pool(name="pA", bufs=1)
        pA = pA_cm.__enter__()
        fullT = pA.tile([D, BC, FT], bf, tag="fullT")
        dma(out=fullT[:, :, 0:TAU], in_=st0[:].rearrange("d (b t) -> d b t", b=BC))
        for b_ in range(BC):
            nc.vector.tensor_copy(fullT[:, b_, TAU:FT],
                                  xT[:, b_ * T:(b_ + 1) * T])

        # ---------- LN (feature-major; stats via ones-matmul) ----------
        def layer_norm(src, ntok_, chunks, grow, brow, ngrow, out_pool, name):
            out_t = out_pool.tile([D, ntok_], bf, tag=f"ln_{name}")
            with tc.tile_pool(name=f"pln_{name}", bufs=1) as pln, \
                 tc.tile_pool(name=f"ps_ln_{name}", bufs=2, space="PSUM") as psp, \
                 tc.tile_pool(name=f"ps_bc_{name}", bufs=2, space="PSUM") as psb:
                sq = pln.tile([D, ntok_], bf, tag="sq")
                mu_t = pln.tile([1, ntok_], f32, tag="mu")
                msq_t = pln.tile([1, ntok_], f32, tag="msq")
                var_t = pln.tile([1, ntok_], f32, tag="var")
                sdt_t = pln.tile([1, ntok_], f32, tag="sdt")
                rstd_t = pln.tile([1, ntok_], f32, tag="rstd")
                musr_t = pln.tile([1, ntok_], f32, tag="musr")
                for ci, (c0, cn) in enumerate(chunks):
                    cs = slice(c0, c0 + cn)
                    warm(3)
                    nc.scalar.square(sq[:, cs], src[:, cs])
                    ps_sum = psp.tile([1, 512], f32, tag="s")
                    ps_sq = psp.tile([1, 512], f32, tag="s")
                    nc.tensor.matmul(ps_sum[:, 0:cn], ones128[0:D, :],
                                     src[:, cs], start=True, stop=True)
                    nc.tensor.matmul(ps_sq[:, 0:cn], ones128[0:D, :],
                                     sq[:, cs], start=True, stop=True)
                    nc.vector.tensor_scalar_mul(mu_t[:, cs], ps_sum[:, 0:cn],
                                                1.0 / D)
                    nc.vector.scalar_tensor_tensor(msq_t[:, cs], mu_t[:, cs], -1.0,
                                                   mu_t[:, cs], OP.mult, OP.mult)
                    nc.vector.scalar_tensor_tensor(var_t[:, cs], ps_sq[:, 0:cn],
                                                   1.0 / D, msq_t[:, cs],
                                                   OP.mult, OP.add)
                    nc.scalar.activation(sdt_t[:, cs], var_t[:, cs], AF.Sqrt,
                                         bias=ceps[:])
                    nc.vector.reciprocal(rstd_t[:, cs], sdt_t[:, cs])
                    nc.vector.scalar_tensor_tensor(musr_t[:, cs], mu_t[:, cs], 0.0,
                                                   rstd_t[:, cs], OP.add, OP.mult)
                    ps_a = psb.tile([D, 512], f32, tag="a")
                    ps_b = psb.tile([D, 512], f32, tag="b")
                    nc.tensor.matmul(ps_a[:, 0:cn], grow,
                                     rstd_t[:, cs], start=True, stop=True)
                    nc.tensor.matmul(ps_b[:, 0:cn], brow,
                                     ones_row[:, cs], start=True, stop=False)
                    nc.tensor.matmul(ps_b[:, 0:cn], ngrow,
                                     musr_t[:, cs], start=False, stop=True)
                    nc.vector.tensor_mul(out_t[:, cs], src[:, cs], ps_a[:, 0:cn])
                    nc.vector.tensor_add(out_t[:, cs], out_t[:, cs], ps_b[:, 0:cn])
            return out_t

        hinT = layer_norm(fullT[:].rearrange("d b t -> d (b t)"), NFTOK, CH3,
                          g1_row, b1_row, ng1_row[:], pA, "1")

        # ---------- qkv ----------
        kT = pA.tile([D, NFTOK], bf, tag="kT")
        q1T = pA.tile([D, NFTOK], bf, tag="q1T")
        q2T = pA.tile([D, NFTOK], bf, tag="q2T")
        vtokA = pA.tile([128, BC, D], bf, tag="vtokA")
        vtokB = pA.tile([FT - 128, BC, D], bf, tag="vtokB")
        with tc.tile_pool(name="ps_qk", bufs=2, space="PSUM") as psqk, \
             tc.tile_pool(name="ps_vt", bufs=2, space="PSUM") as psvt, \
             tc.tile_pool(name="ps_rt", bufs=1, space="PSUM") as psrt:
            for ci, (c0, cn) in enumerate(CH3):
                ps = psqk.tile([128, 512], f32)
                nc.tensor.matmul(ps[:, 0:cn], wqkv_sb[:, 0:128], hinT[:, c0:c0 + cn],
                                 start=True, stop=True)
                nc.scalar.activation(q1T[:, c0:c0 + cn], ps[0:D, 0:cn], AF.Identity,
                                     bias=u_cs[:], scale=INV_SQRT_HD)
                nc.scalar.activation(q2T[:, c0:c0 + cn], ps[0:D, 0:cn], AF.Identity,
                                     bias=v_cs[:], scale=INV_SQRT_HD)
                nc.vector.tensor_copy(kT[:, c0:c0 + cn], ps[D:2 * D, 0:cn])
            for b in range(BC):
                psv = psvt.tile([128, D], f32, tag="va")
                nc.tensor.matmul(psv[:], hinT[:, b * FT:b * FT + 128],
                                 wqkv_sb[:, 128:192], start=True, stop=True)
                nc.vector.tensor_copy(vtokA[:, b, :], psv[:])
                psv2 = psvt.tile([FT - 128, D], f32, tag="vb")
                nc.tensor.matmul(psv2[:], hinT[:, b * FT + 128:(b + 1) * FT],
                                 wqkv_sb[:, 128:192], start=True, stop=True)
                nc.vector.tensor_copy(vtokB[:, b, :], psv2[:])
            ps_rt = psrt.tile([D, FT], f32)
            nc.tensor.matmul(ps_rt[:], wpos_sb, relpos_sb, start=True, stop=True)
            RT = pA.tile([D, FT], bf, tag="RT")
            nc.vector.tensor_copy(RT[:], ps_rt[:])

        # ---------- pos (rel-shift via DRAM bounce) ----------
        with tc.tile_pool(name="pdram", bufs=1, space="DRAM") as pdram:
            posd = pdram.tile([2 * BC, T, FT], bf)
            pos_stage = pA.tile([T, 2 * BC, FT], bf, tag="poss")
            shift_st = pA.tile([T, 2 * BC, FT], bf, tag="shifts")
            with tc.tile_pool(name="ps_pos", bufs=4, space="PSUM") as psp:
                for b in range(BC):
                    for h in range(H):
                        r0 = 32 * h
                        pp = psp.tile([T, FT], f32)
                        nc.tensor.matmul(pp[:],
                                         q2T[r0:r0 + 32, b * FT + TAU:(b + 1) * FT],
                                         RT[r0:r0 + 32, :], start=True, stop=True)
                        nc.scalar.copy(pos_stage[:, 2 * b + h, :], pp[:])
            pd = posd[:]
            for gq in range(4):
                dma(out=posd[4 * gq:4 * gq + 4, :, :].transpose([1, 0, 2]),
                    in_=pos_stage[:, 4 * gq:4 * gq + 4, :])
                shift_src = bass.AP(pd.tensor,
                                    pd.offset + 4 * gq * T * FT + 99,
                                    [[FT - 1, T], [T * FT, 4], [1, FT]])
                dma(out=shift_st[:, 4 * gq:4 * gq + 4, :], in_=shift_src)

        # ---------- attention scores / softmax / av ----------
        avT = pA.tile([D, NTOK], bf, tag="avT")
        rsum16 = pA.tile([T, 2 * BC], f32, tag="rsum16")
        scl16 = pA.tile([T, 2 * BC], f32, tag="scl16")
        wtA = pA.tile([128, 2 * BC, T], bf, tag="wtA")
        wtB = pA.tile([FT - 128, 2 * BC, T], bf, tag="wtB")
        with tc.tile_pool(name="patt", bufs=4) as patt, \
             tc.tile_pool(name="ps_sc", bufs=3, space="PSUM") as ps_scp, \
             tc.tile_pool(name="ps_wt", bufs=2, space="PSUM") as ps_wtp:
            for b in range(BC):
                mb = patt.tile([T, FT], bf, tag="mb")
                nc.vector.tensor_scalar(mb[:], causal_sb, valid_sb[:, b:b + 1],
                                        None, OP.mult)
                for h in range(H):
                    r0 = 32 * h
                    bh = 2 * b + h
                    ps_sc = ps_scp.tile([T, FT], f32)
                    nc.tensor.matmul(ps_sc[:],
                                     q1T[r0:r0 + 32, b * FT + TAU:(b + 1) * FT],
                                     kT[r0:r0 + 32, b * FT:(b + 1) * FT],
                                     start=True, stop=True)
                    s2 = patt.tile([T, FT], f32, tag="s2")
                    nc.vector.tensor_add(s2[:], ps_sc[:], shift_st[:, bh, :])
                    ee = patt.tile([T, FT], bf, tag="ee")
                    nc.scalar.activation(ee[:], s2[:], AF.Exp)
                    ww = patt.tile([T, FT], bf, tag="ww")
                    nc.vector.scalar_tensor_tensor(ww[:], ee[:], 0.0, mb[:],
                                                   OP.add, OP.mult,
                                                   accum_out=rsum16[:, bh:bh + 1])
                    ps_wta = ps_wtp.tile([128, T], bf, tag="wa")
                    ps_wtb = ps_wtp.tile([FT - 128, T], bf, tag="wb")
                    nc.tensor.transpose(ps_wta[:], ww[:, 0:128], ident[0:T, 0:T])
                    nc.tensor.transpose(ps_wtb[:], ww[:, 128:FT], ident[0:T, 0:T])
                    if bh % 2 == 0:
                        nc.vector.tensor_copy(wtA[:, bh, :], ps_wta[:])
                        nc.scalar.copy(wtB[:, bh, :], ps_wtb[:])
                    else:
                        nc.scalar.copy(wtA[:, bh, :], ps_wta[:])
                        nc.vector.tensor_copy(wtB[:, bh, :], ps_wtb[:])

            # batched reciprocal + mask scale
            nc.vector.tensor_scalar_add(scl16[:], rsum16[:], 1e-30)
            nc.vector.reciprocal(scl16[:], scl16[:])
            nc.vector.tensor_mul(
                scl16[:].rearrange("t (b h) -> t b h", b=BC),
                scl16[:].rearrange("t (b h) -> t b h", b=BC),
                avmask_sb.unsqueeze(2).broadcast_to([T, BC, H]))
        with tc.tile_pool(name="patt2", bufs=3) as patt2, \
             tc.tile_pool(name="ps_av", bufs=2, space="PSUM") as ps_avp, \
             tc.tile_pool(name="ps_avt", bufs=2, space="PSUM") as ps_avtp:
            for b in range(BC):
                for h in range(H):
                    r0 = 32 * h
                    bh = 2 * b + h
                    ps_av = ps_avp.tile([T, 32], f32)
                    nc.tensor.matmul(ps_av[:], wtA[:, bh, :], vtokA[:, b, r0:r0 + 32],
                                     start=True, stop=False)
                    nc.tensor.matmul(ps_av[:], wtB[:, bh, :], vtokB[:, b, r0:r0 + 32],
                                     start=False, stop=True)
                    av_sb = patt2.tile([T, 32], bf, tag="avs")
                    nc.scalar.activation(av_sb[:], ps_av[:], AF.Copy,
                                         scale=scl16[:, bh:bh + 1])
                    ps_avt = ps_avtp.tile([32, T], bf)
                    nc.tensor.transpose(ps_avt[:], av_sb[:], ident[0:T, 0:T])
                    nc.vector.tensor_copy(avT[r0:r0 + 32, b * T:(b + 1) * T],
                                          ps_avt[:])

        # ---------- fused matmul + activation helper ----------
        def mm64(dst_tile, lhs_list, rhs_list, act_func, bias=None, psname="m"):
            mout = lhs_list[0].shape[-1]
            with tc.tile_pool(name=f"ps_{psname}", bufs=2, space="PSUM") as psp:
                for ci, (c0, cn) in enumerate(CH):
                    ps = psp.tile([mout, 512], f32)
                    for li, (lh, rh) in enumerate(zip(lhs_list, rhs_list)):
                        nc.tensor.matmul(ps[:, 0:cn], lh, rh[:, c0:c0 + cn],
                                         start=(li == 0),
                                         stop=(li == len(lhs_list) - 1))
                    warm(3)
                    kw = {"bias": bias} if bias is not None else {}
                    nc.scalar.activation(dst_tile[:, c0:c0 + cn], ps[:, 0:cn],
                                         act_func, **kw)
            return dst_tile

        yT = pbig.tile([D, NTOK], bf, tag="yT")
        mm64(yT, [wattn_sb], [avT], AF.Relu, psname="ao")
        pA_cm.__exit__(None, None, None)

        def gru(hT, xgT, gw_sb, negbz, name, out_ap=None):
            out = None
            if out_ap is None:
                out = pbig.tile([D, NTOK], bf, tag=f"o_{name}")
            with tc.tile_pool(name=f"pg_{name}", bufs=1) as pg:
                rt = pg.tile([D, NTOK], bf, tag="r")
                zt = pg.tile([D, NTOK], bf, tag="z")
                ht = pg.tile([D, NTOK], bf, tag="hh")
                xr = pg.tile([D, NTOK], bf, tag="xr")
                mm64(rt, [gw_sb[:, 0, :], gw_sb[:, 1, :]], [xgT, hT], AF.Sigmoid,
                     psname=f"r{name}")
                mm64(zt, [gw_sb[:, 2, :], gw_sb[:, 3, :]], [xgT, hT], AF.Sigmoid,
                     bias=negbz[:], psname=f"z{name}")
                for c0, cn in CH:
                    cs = slice(c0, c0 + cn)
                    warm(2)
                    nc.vector.tensor_mul(xr[:, cs], hT[:, cs], rt[:, cs])
                mm64(ht, [gw_sb[:, 4, :], gw_sb[:, 5, :]], [xgT, xr], AF.Tanh,
                     psname=f"h{name}")
                for c0, cn in CH:
                    cs = slice(c0, c0 + cn)
                    dst = (out_ap if out_ap is not None else out[:, :])
                    nc.vector.tensor_sub(ht[:, cs], ht[:, cs], hT[:, cs])
                    nc.vector.tensor_mul(ht[:, cs], ht[:, cs], zt[:, cs])
                    nc.vector.tensor_add(dst[:, cs], ht[:, cs], hT[:, cs])
            return out

        x1T = gru(xT, yT, g1w_sb, negbz1, "1")
        n2T = layer_norm(x1T[:, :], NTOK, CH, g2_row, b2_row, ng2_row[:], pbig, "2")
        eT = pbig.tile([D, NTOK], bf, tag="eT")
        with tc.tile_pool(name="pmlp", bufs=1) as pmlp:
            mT = pmlp.tile([MLP_D, NTOK], bf, tag="mT")
            mm64(mT, [we1_sb], [n2T], AF.Relu, psname="e1")
            mm64(eT, [we2_sb], [mT], AF.Relu, psname="e2")
        x2aug = pbig.tile([D + 1, NTOK], bf, tag="x2aug")
        nc.vector.memset(x2aug[D:D + 1, :], 1.0)
        gru(x1T, eT, g2w_sb, negbz2, "2", out_ap=x2aug[0:D, :])
        x2T = x2aug

        # ---------- voxel_mean out ----------
        MCH = [(128 * i, 128) for i in range(6)] + [(768, 32)]
        with tc.tile_pool(name="pvst", bufs=2) as pvst, \
             tc.tile_pool(name="ps_vo", bufs=4, space="PSUM") as ps_vop:
            VCH = [(512 * i, 512) for i in range(7)] + [(3584, 416)]
            HALF = [(0, 4, 2048), (4, 8, 1952)]
            for mi, (m0, mn) in enumerate(MCH):
                for hi, (na, nb, hw) in enumerate(HALF):
                    stg = pvst.tile([128, hw], f32, tag=f"vstg{hi}")
                    hbase = VCH[na][0]
                    for ni in range(na, nb):
                        v0, vn = VCH[ni]
                        ps = ps_vop.tile([128, 512], f32)
                        nc.tensor.matmul(ps[0:mn, 0:vn], x2aug[:, m0:m0 + mn],
                                         wvoxb[:, v0:v0 + vn],
                                         start=True, stop=True)
                        if ni % 2 == 0:
                            nc.scalar.copy(stg[0:mn, v0 - hbase:v0 - hbase + vn],
                                           ps[0:mn, 0:vn])
                        else:
                            nc.vector.tensor_copy(
                                stg[0:mn, v0 - hbase:v0 - hbase + vn],
                                ps[0:mn, 0:vn])
                    dma(out=ovox[m0:m0 + mn, hbase:hbase + hw], in_=stg[0:mn, :])

        # ---------- heads ----------
        with tc.tile_pool(name="ps_hd", bufs=1, space="PSUM") as ps_hd, \
             tc.tile_pool(name="phd", bufs=1) as phd:
            selb_ps = ps_hd.tile([D, 2, 512], f32, tag="selb")
            seled = phd.tile([D, NTOK], bf, tag="seled")
            for ci, (c0, cn) in enumerate(CH):
                nc.tensor.matmul(selb_ps[:, ci, 0:cn], ones1x64[:],
                                 sel_sb[:, c0:c0 + cn], start=True, stop=True)
                nc.vector.tensor_mul(seled[:, c0:c0 + cn], x2T[0:D, c0:c0 + cn],
                                     selb_ps[:, ci, 0:cn])
            lastT = phd.tile([D, BC], f32, tag="lastT")
            nc.vector.tensor_reduce(lastT[:],
                                    seled[:].rearrange("d (b t) -> d b t", b=BC),
                                    axis=AX.X, op=OP.add)
            lastb = phd.tile([D, BC], bf, tag="lastb")
            nc.vector.tensor_copy(lastb[:], lastT[:])
            ps_v1 = ps_hd.tile([D, BC], f32, tag="v1")
            nc.tensor.matmul(ps_v1[:], wval1_sb, lastb[:], start=True, stop=True)
            v1 = phd.tile([D, BC], bf, tag="v1s")
            nc.scalar.activation(v1[:], ps_v1[:], AF.Relu, bias=bval1_c)
            ps_vv = ps_hd.tile([1, BC], f32, tag="vv")
            nc.tensor.matmul(ps_vv[:], wval2_sb, v1[:], start=True, stop=True)
            valo = phd.tile([1, BC], f32, tag="valo")
            nc.scalar.activation(valo[:], ps_vv[:], AF.Identity, bias=bval2_c)
            dma(out=oval[:], in_=valo[:])
            ps_a1 = ps_hd.tile([D, BC], f32, tag="a1")
            nc.tensor.matmul(ps_a1[:], wa1_sb, lastb[:], start=True, stop=True)
            a1 = phd.tile([D, BC], bf, tag="a1s")
            nc.scalar.activation(a1[:], ps_a1[:], AF.Relu, bias=ba1_c)
            ps_ao = ps_hd.tile([NOUT, BC], f32, tag="aco")
            nc.tensor.matmul(ps_ao[:], wa2_sb, a1[:], start=True, stop=True)
            acto = phd.tile([NOUT, BC], f32, tag="acto")
            nc.scalar.activation(acto[:], ps_ao[:], AF.Identity, bias=ba2_c)
            dma(out=oact[:], in_=acto[:])

    nc.compile()
    return nc


def _prep_inputs(gaussians, gaussian_num, all_past_voxels, state0, params):
    gaussians = np.asarray(gaussians, np.float32)
    lengths = np.asarray(gaussian_num).astype(np.int64)
    vox_full = np.asarray(all_past_voxels)
    state0 = np.asarray(state0, np.float32)
    P = {k: (np.asarray(v, np.float32) if not isinstance(v, dict) else
             {k2: np.asarray(v2, np.float32) for k2, v2 in v.items()})
         for k, v in params.items()}
    minlen = int(lengths.min())
    ii = np.arange(T)

    b64 = np.zeros((D, NB64), np.float32)

    def put(name, arr):
        o, w = _B64[name]
        arr = np.asarray(arr, np.float32)
        if arr.ndim == 1:
            arr = arr[:, None]
        b64[:arr.shape[0], o:o + w] = arr

    put("wqkv", P["w_qkv"]); put("wpos", P["w_pos"]); put("wattn", P["w_attn"])
    put("g1w", np.concatenate([P["gru1"][k] for k in
                               ("wr", "ur", "wz", "uz", "wh", "uh")], axis=1))
    put("g2w", np.concatenate([P["gru2"][k] for k in
                               ("wr", "ur", "wz", "uz", "wh", "uh")], axis=1))
    put("we1", P["w_e1"]); put("we2", P["w_e2"])
    put("wa1", P["w_a1"]); put("wa2", P["w_a2"])
    put("wval1", P["w_val1"]); put("wval2", P["w_val2"])
    put("relposT", _relpos_table())
    put("ba1", P["b_a1"]); put("bval1", P["b_val1"])
    put("ba2", P["b_a2"]); put("bval2", P["b_val2"].reshape(1))
    put("g1bz", P["gru1"]["bz"]); put("g2bz", P["gru2"]["bz"])
    put("u", P["uvar"].reshape(D)); put("v", P["vvar"].reshape(D))
    put("wg", np.concatenate([P["w_g"], P["b_g"][None, :]], 0))

    blobr = np.zeros((1, NBR), np.float32)
    blobr[0, 0:64] = P["ln1_g"]; blobr[0, 64:128] = P["ln1_b"]
    blobr[0, 128:192] = P["ln2_g"]; blobr[0, 192:256] = P["ln2_b"]
    blobr[0, 256:304] = P["b_v"]

    causal = (np.arange(FT)[None, :] < (TAU + 1 + ii)[:, None]).astype(np.float32)
    wvm = np.ascontiguousarray(
        P["w_v"].reshape(NCELL, NMAT, VFEAT).transpose(1, 0, 2), np.float32)

    in_maps = []
    for c in range(NCORES):
        sl = slice(c * BC, (c + 1) * BC)
        g = gaussians[sl, :T].reshape(BC * T, GF).T
        gaug = np.concatenate([g, np.ones((1, NTOK), np.float32)], 0)
        v = vox_full[sl, :T].reshape(BC, T, NCELL).copy()
        v[:, 0] = 0
        voxT = np.ascontiguousarray(v.reshape(NTOK, NCELL).T.astype(np.int8))
        st = np.ascontiguousarray(state0[sl].reshape(BC * TAU, D).T)
        c0r = np.ones(NTOK, np.float32)
        c0r[0::T] = 0.0
        a2 = np.ascontiguousarray(np.stack([np.ones(NTOK, np.float32), c0r]))
        ln = lengths[sl]
        valid = (ii[:, None] <= ln[None, :]).astype(np.float32)       # [100, 8]
        avm = valid * (ii[:, None] < TAU + 1 + minlen).astype(np.float32)
        selr = np.zeros((1, NTOK), np.float32)
        for b in range(BC):
            selr[0, b * T + int(ln[b])] = 1.0
        b100 = np.zeros((T, NB100), np.float32)
        b100[:, 0:BC] = valid
        b100[:, BC:2 * BC] = avm
        b100[:, 2 * BC:2 * BC + FT] = causal

        m = {"vox": voxT, "gaus": np.ascontiguousarray(gaug).astype(BF16),
             "st0": st.astype(BF16), "aug2": a2.astype(BF16),
             "sel": selr.astype(BF16),
             "blob64": b64.astype(BF16), "blob100": b100, "blobr": blobr,
             "wvm": wvm.astype(BF16), "wvox": P["w_vox"].astype(BF16),
             "bvox": P["b_vox"].astype(BF16),
             "bvr": P["b_v"].reshape(1, VFEAT).astype(BF16)}
        in_maps.append(m)
    return in_maps


def kernel(gaussians, gaussian_num, all_past_voxels, state0, params):
    from concourse.bass_utils import run_bass_kernel_spmd
    if "nc" not in _CACHE:
        _CACHE["nc"] = _build()
    nc = _CACHE["nc"]
    in_maps = _prep_inputs(gaussians, gaussian_num, all_past_voxels, state0, params)
    res = run_bass_kernel_spmd(nc, in_maps, core_ids=list(range(NCORES)))
    acts, voxs, vals = [], [], []
    for c in range(NCORES):
        r = res.results[c]
        acts.append(np.ascontiguousarray(r["oact"].T))
        voxs.append(r["ovox"].reshape(BC, T, VDIM))
        vals.append(r["oval"].reshape(BC))
    act = np.concatenate(acts, 0).astype(np.float32)
    voxm = np.concatenate(voxs, 0).astype(np.float32)
    val = np.concatenate(vals, 0).astype(np.float32)
    return act, voxm, val


# revision 31
# speedup vs baseline: 1.0345x; 1.0345x over previous
"""Trainium2 Bass kernel for nn_Actor (ragged GTrXL-style actor network).

Pure data parallel over 8 NeuronCores: 8 samples per core. The full forward
(one-hot voxel featurization via a hinge-basis decomposition, rel-pos
attention with ragged masks, two GRU gates, LN/MLP, voxel-mean head,
value/action heads) runs on-device in a single NEFF; the host only shards
inputs / reassembles outputs.

Self-contained: hardcodes all shapes; no sibling imports.
"""
import sys

if "/opt/trn_rl_repo" not in sys.path:
    sys.path.insert(0, "/opt/trn_rl_repo")

import numpy as np
import ml_dtypes
BF16 = ml_dtypes.bfloat16

# ---- problem constants ----
B, T_OBS, GF = 64, 101, 7
T = 100
NCELL, NMAT = 1000, 4
VDIM = NCELL * NMAT          # 4000
D, H, HD = 64, 2, 32
GDIM, VFEAT = 16, 48
TAU = 50
MLP_D = 32
NOUT = 16
EPS = 1e-5
NCORES = 8
BC = B // NCORES             # 8 samples / core
NTOK = BC * T                # 800 tokens / core
FT = TAU + T                 # 150
NFTOK = BC * FT              # 1200
INV_SQRT_HD = 1.0 / np.sqrt(HD)

# blob64 column layout
_B64 = {}
_off = 0
for _name, _w in [("wqkv", 192), ("wpos", 64), ("wattn", 64), ("g1w", 384),
                  ("g2w", 384), ("we1", 32), ("we2", 64), ("wa1", 64),
                  ("wa2", 16), ("wval1", 64), ("wval2", 1), ("relposT", 150),
                  ("ba1", 1), ("bval1", 1), ("ba2", 1), ("bval2", 1),
                  ("g1bz", 1), ("g2bz", 1), ("u", 1), ("v", 1), ("wg", 16)]:
    _B64[_name] = (_off, _w)
    _off += _w
NB64 = _off
# blob100 column layout: valid 0:8 | avmask 8:16 | causal 16:166
NB100 = 166
# blobR: ln1g 0:64 | ln1b 64:128 | ln2g 128:192 | ln2b 192:256 | bv 256:304
NBR = 304

_CACHE = {}


def _relpos_table():
    inv = 1.0 / (10000.0 ** (np.arange(0, D, 2, dtype=np.float32) / D))
    pos = np.arange(FT - 1, -1, -1, dtype=np.float32)
    ang = pos[:, None] * inv[None, :]
    return np.concatenate([np.sin(ang), np.cos(ang)], -1).T.copy()  # [64, 150]


def _build():
    import concourse.bass as bass
    import concourse.tile as tile
    from concourse import bacc, mybir
    from concourse.masks import make_identity
    from contextlib import ExitStack

    f32 = mybir.dt.float32
    bf = mybir.dt.bfloat16
    i8 = mybir.dt.int8
    AF = mybir.ActivationFunctionType
    OP = mybir.AluOpType
    AX = mybir.AxisListType

    nc = bacc.Bacc("TRN2", target_bir_lowering=False, debug=False,
                   enable_asserts=True, num_devices=NCORES)

    def din(name, shape, dt=f32):
        return nc.declare_dram_parameter(name, list(shape), dt, isOutput=False)

    # inputs
    vox = din("vox", (NCELL, NTOK), i8)
    gaus = din("gaus", (GF + 1, NTOK), bf)
    st0 = din("st0", (D, BC * TAU), bf)
    aug2 = din("aug2", (2, NTOK), bf)
    sel = din("sel", (1, NTOK), bf)
    blob64 = din("blob64", (D, NB64), bf)
    blob100 = din("blob100", (T, NB100))
    blobr = din("blobr", (1, NBR))
    wvm = din("wvm", (NMAT, NCELL, VFEAT), bf)
    wvox = din("wvox", (D, VDIM), bf)
    bvox = din("bvox", (VDIM,), bf)
    bvr = din("bvr", (1, VFEAT), bf)
    # outputs
    ovox = nc.declare_dram_parameter("ovox", [NTOK, VDIM], f32, isOutput=True)
    oact = nc.declare_dram_parameter("oact", [NOUT, BC], f32, isOutput=True)
    oval = nc.declare_dram_parameter("oval", [1, BC], f32, isOutput=True)

    KT = [128] * 7 + [104]                          # cell tiles
    CH = [(0, 512), (512, 288)]                     # NTOK chunks (<=512)
    CH3 = [(0, 512), (512, 512), (1024, 176)]       # NFTOK chunks

    with tile.TileContext(nc) as tc, ExitStack() as ctx:
        dma = nc.sync.dma_start

        pconst = ctx.enter_context(tc.tile_pool(name="pconst", bufs=1))
        pbig = ctx.enter_context(tc.tile_pool(name="pbig", bufs=1))
        pwarm = ctx.enter_context(tc.tile_pool(name="pwarm", bufs=1, space="PSUM"))
        warm_ps = pwarm.tile([D, D], f32, tag="warm")
        warm_src = [None]

        def warm(n=2, anchor=None):
            if warm_src[0] is None:
                return
            for _ in range(n):
                nc.tensor.matmul(warm_ps[0:64, 0:64], warm_src[0], warm_src[0],
                                 start=True, stop=True)

        # ---------- early bulk input DMAs (longest supply chains) ----------
        vox_tiles = []
        KT0 = [128] * 7 + [104]
        for k in range(8):
            vt_ = pconst.tile([128, NTOK], i8, tag=f"vt{k}")
            dma(out=vt_[0:KT0[k], :], in_=vox[128 * k:128 * k + KT0[k], :])
            vox_tiles.append(vt_)
        wvm_tiles = []
        for m in range(NMAT):
            t_ = pconst.tile([128, 8, VFEAT], bf, tag=f"wm{m}")
            nc.vector.memset(t_[96:128, 7, :], 0.0)
            dma(out=t_[:, 0:7, :],
                in_=wvm[m, 0:896, :].rearrange("(k p) f -> p k f", p=128))
            dma(out=t_[0:104, 7, :], in_=wvm[m, 896:1000, :])
            wvm_tiles.append(t_)

        # ---------- constants / weights into SBUF ----------
        ident = pconst.tile([128, 128], bf, tag="ident")
        make_identity(nc, ident[:])

        b64 = pconst.tile([D, NB64], bf, tag="b64")
        wg_aug = None
        dma(out=b64[:], in_=blob64[:])

        def b64s(name):
            o, w = _B64[name]
            return b64[:, o:o + w]

        b100 = pconst.tile([T, NB100], f32, tag="b100")
        dma(out=b100[:], in_=blob100[:])
        valid_sb = b100[:, 0:BC]
        avmask_sb = b100[:, BC:2 * BC]
        causal_sb = b100[:, 2 * BC:2 * BC + FT]


        br = pconst.tile([1, NBR], f32, tag="br")
        dma(out=br[:], in_=blobr[:])

        gaus_sb = pconst.tile([GF + 1, NTOK], bf, tag="gaus")
        dma(out=gaus_sb[:], in_=gaus[:])
        aug_ones = pconst.tile([1, NTOK], bf, tag="augo")
        dma(out=aug_ones[:], in_=aug2[0, :].unsqueeze(0))
        aug_c0 = pconst.tile([1, NTOK], bf, tag="augc")
        dma(out=aug_c0[:], in_=aug2[1, :].unsqueeze(0))
        sel_sb = pconst.tile([1, NTOK], bf, tag="sel")
        dma(out=sel_sb[:], in_=sel[:])

        warm_src[0] = b64s("wqkv")[:, 0:64]
        wqkv_sb = b64s("wqkv")
        wpos_sb = b64s("wpos")
        wattn_sb = b64s("wattn")
        g1w_sb = b64s("g1w").rearrange("d (i m) -> d i m", i=6)
        g2w_sb = b64s("g2w").rearrange("d (i m) -> d i m", i=6)
        we1_sb = b64s("we1")
        we2_sb = b64s("we2")[0:MLP_D, :]
        wa1_sb = b64s("wa1")
        wa2_sb = b64s("wa2")
        wval1_sb = b64s("wval1")
        wval2_sb = b64s("wval2")
        relpos_sb = b64s("relposT")
        wg_aug = b64s("wg")[0:GF + 1, :]
        ba1_c = b64s("ba1")
        bval1_c = b64s("bval1")
        ba2_c = b64s("ba2")[0:NOUT, :]
        bval2_c = b64s("bval2")[0:1, :]

        u_cs = pconst.tile([D, 1], f32, tag="ucs")
        v_cs = pconst.tile([D, 1], f32, tag="vcs")
        nc.vector.tensor_scalar_mul(u_cs[:], b64s("u"), INV_SQRT_HD)
        nc.vector.tensor_scalar_mul(v_cs[:], b64s("v"), INV_SQRT_HD)
        negbz1 = pconst.tile([D, 1], f32, tag="nbz1")
        negbz2 = pconst.tile([D, 1], f32, tag="nbz2")
        nc.vector.tensor_scalar_mul(negbz1[:], b64s("g1bz"), -1.0)
        nc.vector.tensor_scalar_mul(negbz2[:], b64s("g2bz"), -1.0)

        g1_row = br[:, 0:64]
        b1_row = br[:, 64:128]
        g2_row = br[:, 128:192]
        b2_row = br[:, 192:256]
        ng1_row = pconst.tile([1, D], f32, tag="ng1")
        ng2_row = pconst.tile([1, D], f32, tag="ng2")
        nc.vector.tensor_scalar_mul(ng1_row[:], g1_row, -1.0)
        nc.vector.tensor_scalar_mul(ng2_row[:], g2_row, -1.0)
        ones_row = pconst.tile([1, NFTOK], f32, tag="onesrow")
        nc.vector.memset(ones_row[:], 1.0)
        ones128 = pconst.tile([128, 1], bf, tag="ones128")
        nc.vector.memset(ones128[:], 1.0)
        ones1x64 = pconst.tile([1, D], bf, tag="ones1x64")
        nc.vector.memset(ones1x64[:], 1.0)
        cm2 = pconst.tile([128, 1], f32, tag="cm2")
        nc.vector.memset(cm2[:], -2.0)
        ceps = pconst.tile([1, 1], f32, tag="ceps")
        nc.vector.memset(ceps[:], EPS)

        wvoxb = pconst.tile([D + 1, VDIM], bf, tag="wvoxb")
        dma(out=wvoxb[0:D, :], in_=wvox[:])
        dma(out=wvoxb[D:D + 1, :], in_=bvox[:].unsqueeze(0))

        # ---------- phase V: voxel hinge-basis features ----------
        W0, W1, W2, W3 = wvm_tiles
        C1 = pconst.tile([128, 8, VFEAT], bf, tag="c1")
        C2 = pconst.tile([128, 8, VFEAT], bf, tag="c2")
        C3 = pconst.tile([128, 8, VFEAT], bf, tag="c3")
        tmpC = pconst.tile([128, 8, VFEAT], bf, tag="tmpc")
        nc.vector.tensor_sub(C1[:], W1[:], W0[:])
        nc.vector.scalar_tensor_tensor(tmpC[:], W1[:], -2.0, W2[:], OP.mult, OP.add)
        nc.vector.tensor_add(C2[:], tmpC[:], W0[:])
        nc.vector.scalar_tensor_tensor(C3[:], W2[:], -2.0, W3[:], OP.mult, OP.add)
        nc.vector.tensor_add(C3[:], C3[:], W1[:])
        CB = [C1, C2, C3]

        with tc.tile_pool(name="ps_w0", bufs=1, space="PSUM") as ps_w0:
            ps_sw0 = ps_w0.tile([1, VFEAT], f32)
            for k in range(8):
                nc.tensor.matmul(ps_sw0[:], ones128[0:KT[k], :], W0[0:KT[k], k, :],
                                 start=(k == 0), stop=(k == 7))
            sw0_stage = pconst.tile([1, VFEAT], bf, tag="sw0")
            nc.vector.tensor_copy(sw0_stage[:], ps_sw0[:])
        bvr_sb = pconst.tile([1, VFEAT], bf, tag="bvrs")
        dma(out=bvr_sb[:], in_=bvr[:])

        xT = pbig.tile([D, NTOK], bf, tag="xT")

        with tc.tile_pool(name="pmask", bufs=2) as pmask, \
             tc.tile_pool(name="ps_vf", bufs=1, space="PSUM") as ps_vfp, \
             tc.tile_pool(name="ps_g", bufs=1, space="PSUM") as ps_gp:
            ps_vf = ps_vfp.tile([VFEAT, 2, 512], f32)
            for k in range(8):
                kn = KT[k]
                vtile = vox_tiles[k]
                vf32 = pmask.tile([128, NTOK], bf, tag="m0")
                r1 = pmask.tile([128, NTOK], bf, tag="m1")
                r2 = pmask.tile([128, NTOK], bf, tag="m2")
                warm(2)
                nc.vector.tensor_copy(vf32[0:kn, :], vtile[0:kn, :])
                nc.vector.tensor_scalar(r1[0:kn, :], vf32[0:kn, :], 1.0, 0.0,
                                        OP.subtract, OP.max)
                nc.scalar.activation(r2[0:kn, :], vf32[0:kn, :], AF.Relu,
                                     bias=cm2[0:kn, :])
                for ci, (c0, cn) in enumerate(CH):
                    for bi, mask in enumerate((vf32, r1, r2)):
                        nc.tensor.matmul(ps_vf[:, ci, 0:cn], CB[bi][0:kn, k, :],
                                         mask[0:kn, c0:c0 + cn],
                                         start=(k == 0 and bi == 0), stop=False)
            for ci, (c0, cn) in enumerate(CH):
                nc.tensor.matmul(ps_vf[:, ci, 0:cn], bvr_sb[:],
                                 aug_ones[:, c0:c0 + cn], start=False, stop=False)
                nc.tensor.matmul(ps_vf[:, ci, 0:cn], sw0_stage[:],
                                 aug_c0[:, c0:c0 + cn], start=False, stop=True)
            ps_gt = ps_gp.tile([GDIM, 2, 512], f32)
            vf_stage = pmask.tile([VFEAT, NTOK], bf, tag="vfst")
            for ci, (c0, cn) in enumerate(CH):
                nc.tensor.matmul(ps_gt[:, ci, 0:cn], wg_aug,
                                 gaus_sb[:, c0:c0 + cn], start=True, stop=True)
                nc.scalar.copy(xT[0:GDIM, c0:c0 + cn], ps_gt[:, ci, 0:cn])
                nc.scalar.copy(vf_stage[:, c0:c0 + cn], ps_vf[:, ci, 0:cn])
            for c0, cn in CH:
                dma(out=xT[GDIM:D, c0:c0 + cn], in_=vf_stage[:, c0:c0 + cn])

        # ---------- fullT = [mem | x] per sample ----------
        pA_cm = tc.tile_pool(name="pA", bufs=1)
        pA = pA_cm.__enter__()
        fullT = pA.tile([D, BC, FT], bf, tag="fullT")
        dma(out=fullT[:, :, 0:TAU], in_=st0[:].rearrange("d (b t) -> d b t", b=BC))
        for b_ in range(BC):
            nc.vector.tensor_copy(fullT[:, b_, TAU:FT],
                                  xT[:, b_ * T:(b_ + 1) * T])

        # ---------- LN (feature-major; stats via ones-matmul) ----------
        def layer_norm(src, ntok_, chunks, grow, brow, ngrow, out_pool, name):
            out_t = out_pool.tile([D, ntok_], bf, tag=f"ln_{name}")
            with tc.tile_pool(name=f"pln_{name}", bufs=1) as pln, \
                 tc.tile_pool(name=f"ps_ln_{name}", bufs=2, space="PSUM") as psp, \
                 tc.tile_pool(name=f"ps_bc_{name}", bufs=2, space="PSUM") as psb:
                sq = pln.tile([D, ntok_], bf, tag="sq")
                mu_t = pln.tile([1, ntok_], f32, tag="mu")
                msq_t = pln.tile([1, ntok_], f32, tag="msq")
                var_t = pln.tile([1, ntok_], f32, tag="var")
                sdt_t = pln.tile([1, ntok_], f32, tag="sdt")
                rstd_t = pln.tile([1, ntok_], f32, tag="rstd")
                musr_t = pln.tile([1, ntok_], f32, tag="musr")
                for ci, (c0, cn) in enumerate(chunks):
                    cs = slice(c0, c0 + cn)
                    warm(3)
                    nc.scalar.square(sq[:, cs], src[:, cs])
                    ps_sum = psp.tile([1, 512], f32, tag="s")
                    ps_sq = psp.tile([1, 512], f32, tag="s")
                    nc.tensor.matmul(ps_sum[:, 0:cn], ones128[0:D, :],
                                     src[:, cs], start=True, stop=True)
                    nc.tensor.matmul(ps_sq[:, 0:cn], ones128[0:D, :],
                                     sq[:, cs], start=True, stop=True)
                    nc.vector.tensor_scalar_mul(mu_t[:, cs], ps_sum[:, 0:cn],
                                                1.0 / D)
                    nc.vector.scalar_tensor_tensor(msq_t[:, cs], mu_t[:, cs], -1.0,
                                                   mu_t[:, cs], OP.mult, OP.mult)
                    nc.vector.scalar_tensor_tensor(var_t[:, cs], ps_sq[:, 0:cn],
                                                   1.0 / D, msq_t[:, cs],
                                                   OP.mult, OP.add)
                    nc.scalar.activation(sdt_t[:, cs], var_t[:, cs], AF.Sqrt,
                                         bias=ceps[:])
                    nc.vector.reciprocal(rstd_t[:, cs], sdt_t[:, cs])
                    nc.vector.scalar_tensor_tensor(musr_t[:, cs], mu_t[:, cs], 0.0,
                                                   rstd_t[:, cs], OP.add, OP.mult)
                    ps_a = psb.tile([D, 512], f32, tag="a")
                    ps_b = psb.tile([D, 512], f32, tag="b")
                    nc.tensor.matmul(ps_a[:, 0:cn], grow,
                                     rstd_t[:, cs], start=True, stop=True)
                    nc.tensor.matmul(ps_b[:, 0:cn], brow,
                                     ones_row[:, cs], start=True, stop=False)
                    nc.tensor.matmul(ps_b[:, 0:cn], ngrow,
                                     musr_t[:, cs], start=False, stop=True)
                    nc.vector.tensor_mul(out_t[:, cs], src[:, cs], ps_a[:, 0:cn])
                    nc.vector.tensor_add(out_t[:, cs], out_t[:, cs], ps_b[:, 0:cn])
            return out_t

        hinT = layer_norm(fullT[:].rearrange("d b t -> d (b t)"), NFTOK, CH3,
                          g1_row, b1_row, ng1_row[:], pA, "1")

        # ---------- qkv ----------
        kT = pA.tile([D, NFTOK], bf, tag="kT")
        q1T = pA.tile([D, NFTOK], bf, tag="q1T")
        q2T = pA.tile([D, NFTOK], bf, tag="q2T")
        vtokA = pA.tile([128, BC, D], bf, tag="vtokA")
        vtokB = pA.tile([FT - 128, BC, D], bf, tag="vtokB")
        with tc.tile_pool(name="ps_qk", bufs=2, space="PSUM") as psqk, \
             tc.tile_pool(name="ps_vt", bufs=2, space="PSUM") as psvt, \
             tc.tile_pool(name="ps_rt", bufs=1, space="PSUM") as psrt:
            for ci, (c0, cn) in enumerate(CH3):
                ps = psqk.tile([128, 512], f32)
                nc.tensor.matmul(ps[:, 0:cn], wqkv_sb[:, 0:128], hinT[:, c0:c0 + cn],
                                 start=True, stop=True)
                nc.scalar.activation(q1T[:, c0:c0 + cn], ps[0:D, 0:cn], AF.Identity,
                                     bias=u_cs[:], scale=INV_SQRT_HD)
                nc.scalar.activation(q2T[:, c0:c0 + cn], ps[0:D, 0:cn], AF.Identity,
                                     bias=v_cs[:], scale=INV_SQRT_HD)
                nc.vector.tensor_copy(kT[:, c0:c0 + cn], ps[D:2 * D, 0:cn])
            for b in range(BC):
                psv = psvt.tile([128, D], f32, tag="va")
                nc.tensor.matmul(psv[:], hinT[:, b * FT:b * FT + 128],
                                 wqkv_sb[:, 128:192], start=True, stop=True)
                nc.vector.tensor_copy(vtokA[:, b, :], psv[:])
                psv2 = psvt.tile([FT - 128, D], f32, tag="vb")
                nc.tensor.matmul(psv2[:], hinT[:, b * FT + 128:(b + 1) * FT],
                                 wqkv_sb[:, 128:192], start=True, stop=True)
                nc.vector.tensor_copy(vtokB[:, b, :], psv2[:])
            ps_rt = psrt.tile([D, FT], f32)
            nc.tensor.matmul(ps_rt[:], wpos_sb, relpos_sb, start=True, stop=True)
            RT = pA.tile([D, FT], bf, tag="RT")
            nc.vector.tensor_copy(RT[:], ps_rt[:])

        # ---------- pos (rel-shift via DRAM bounce) ----------
        with tc.tile_pool(name="pdram", bufs=1, space="DRAM") as pdram:
            posd = pdram.tile([2 * BC, T, FT], bf)
            pos_stage = pA.tile([T, 2 * BC, FT], bf, tag="poss")
            shift_st = pA.tile([T, 2 * BC, FT], bf, tag="shifts")
            with tc.tile_pool(name="ps_pos", bufs=4, space="PSUM") as psp:
                for b in range(BC):
                    for h in range(H):
                        r0 = 32 * h
                        pp = psp.tile([T, FT], f32)
                        nc.tensor.matmul(pp[:],
                                         q2T[r0:r0 + 32, b * FT + TAU:(b + 1) * FT],
                                         RT[r0:r0 + 32, :], start=True, stop=True)
                        nc.scalar.copy(pos_stage[:, 2 * b + h, :], pp[:])
            pd = posd[:]
            for gq in range(4):
                dma(out=posd[4 * gq:4 * gq + 4, :, :].transpose([1, 0, 2]),
                    in_=pos_stage[:, 4 * gq:4 * gq + 4, :])
                shift_src = bass.AP(pd.tensor,
                                    pd.offset + 4 * gq * T * FT + 99,
                                    [[FT - 1, T], [T * FT, 4], [1, FT]])
                dma(out=shift_st[:, 4 * gq:4 * gq + 4, :], in_=shift_src)

        # ---------- attention scores / softmax / av ----------
        avT = pA.tile([D, NTOK], bf, tag="avT")
        rsum16 = pA.tile([T, 2 * BC], f32, tag="rsum16")
        scl16 = pA.tile([T, 2 * BC], f32, tag="scl16")
        wtA = pA.tile([128, 2 * BC, T], bf, tag="wtA")
        wtB = pA.tile([FT - 128, 2 * BC, T], bf, tag="wtB")
        with tc.tile_pool(name="patt", bufs=4) as patt, \
             tc.tile_pool(name="ps_sc", bufs=3, space="PSUM") as ps_scp, \
             tc.tile_pool(name="ps_wt", bufs=2, space="PSUM") as ps_wtp:
            for b in range(BC):
                mb = patt.tile([T, FT], bf, tag="mb")
                nc.vector.tensor_scalar(mb[:], causal_sb, valid_sb[:, b:b + 1],
                                        None, OP.mult)
                for h in range(H):
                    r0 = 32 * h
                    bh = 2 * b + h
                    ps_sc = ps_scp.tile([T, FT], f32)
                    nc.tensor.matmul(ps_sc[:],
                                     q1T[r0:r0 + 32, b * FT + TAU:(b + 1) * FT],
                                     kT[r0:r0 + 32, b * FT:(b + 1) * FT],
                                     start=True, stop=True)
                    s2 = patt.tile([T, FT], f32, tag="s2")
                    nc.vector.tensor_add(s2[:], ps_sc[:], shift_st[:, bh, :])
                    ee = patt.tile([T, FT], bf, tag="ee")
                    nc.scalar.activation(ee[:], s2[:], AF.Exp)
                    ww = patt.tile([T, FT], bf, tag="ww")
                    nc.vector.scalar_tensor_tensor(ww[:], ee[:], 0.0, mb[:],
                                                   OP.add, OP.mult,
                                                   accum_out=rsum16[:, bh:bh + 1])
                    ps_wta = ps_wtp.tile([128, T], bf, tag="wa")
                    ps_wtb = ps_wtp.tile([FT - 128, T], bf, tag="wb")
                    nc.tensor.transpose(ps_wta[:], ww[:, 0:128], ident[0:T, 0:T])
                    nc.tensor.transpose(ps_wtb[:], ww[:, 128:FT], ident[0:T, 0:T])
                    if bh % 2 == 0:
                        nc.vector.tensor_copy(wtA[:, bh, :], ps_wta[:])
                        nc.scalar.copy(wtB[:, bh, :], ps_wtb[:])
                    else:
                        nc.scalar.copy(wtA[:, bh, :], ps_wta[:])
                        nc.vector.tensor_copy(wtB[:, bh, :], ps_wtb[:])

            # batched reciprocal + mask scale
            nc.vector.tensor_scalar_add(scl16[:], rsum16[:], 1e-30)
            nc.vector.reciprocal(scl16[:], scl16[:])
            nc.vector.tensor_mul(
                scl16[:].rearrange("t (b h) -> t b h", b=BC),
                scl16[:].rearrange("t (b h) -> t b h", b=BC),
                avmask_sb.unsqueeze(2).broadcast_to([T, BC, H]))
        with tc.tile_pool(name="patt2", bufs=3) as patt2, \
             tc.tile_pool(name="ps_av", bufs=2, space="PSUM") as ps_avp, \
             tc.tile_pool(name="ps_avt", bufs=2, space="PSUM") as ps_avtp:
            for b in range(BC):
                for h in range(H):
                    r0 = 32 * h
                    bh = 2 * b + h
                    ps_av = ps_avp.tile([T, 32], f32)
                    nc.tensor.matmul(ps_av[:], wtA[:, bh, :], vtokA[:, b, r0:r0 + 32],
                                     start=True, stop=False)
                    nc.tensor.matmul(ps_av[:], wtB[:, bh, :], vtokB[:, b, r0:r0 + 32],
                                     start=False, stop=True)
                    av_sb = patt2.tile([T, 32], bf, tag="avs")
                    nc.scalar.activation(av_sb[:], ps_av[:], AF.Copy,
                                         scale=scl16[:, bh:bh + 1])
                    ps_avt = ps_avtp.tile([32, T], bf)
                    nc.tensor.transpose(ps_avt[:], av_sb[:], ident[0:T, 0:T])
                    nc.vector.tensor_copy(avT[r0:r0 + 32, b * T:(b + 1) * T],
                                          ps_avt[:])

        # ---------- fused matmul + activation helper ----------
        def mm64(dst_tile, lhs_list, rhs_list, act_func, bias=None, psname="m"):
            mout = lhs_list[0].shape[-1]
            with tc.tile_pool(name=f"ps_{psname}", bufs=2, space="PSUM") as psp:
                for ci, (c0, cn) in enumerate(CH):
                    ps = psp.tile([mout, 512], f32)
                    for li, (lh, rh) in enumerate(zip(lhs_list, rhs_list)):
                        nc.tensor.matmul(ps[:, 0:cn], lh, rh[:, c0:c0 + cn],
                                         start=(li == 0),
                                         stop=(li == len(lhs_list) - 1))
                    warm(3)
                    kw = {"bias": bias} if bias is not None else {}
                    nc.scalar.activation(dst_tile[:, c0:c0 + cn], ps[:, 0:cn],
                                         act_func, **kw)
            return dst_tile

        yT = pbig.tile([D, NTOK], bf, tag="yT")
        mm64(yT, [wattn_sb], [avT], AF.Relu, psname="ao")
        pA_cm.__exit__(None, None, None)

        def gru(hT, xgT, gw_sb, negbz, name, out_ap=None):
            out = None
            if out_ap is None:
                out = pbig.tile([D, NTOK], bf, tag=f"o_{name}")
            with tc.tile_pool(name=f"pg_{name}", bufs=1) as pg:
                rt = pg.tile([D, NTOK], bf, tag="r")
                zt = pg.tile([D, NTOK], bf, tag="z")
                ht = pg.tile([D, NTOK], bf, tag="hh")
                xr = pg.tile([D, NTOK], bf, tag="xr")
                mm64(rt, [gw_sb[:, 0, :], gw_sb[:, 1, :]], [xgT, hT], AF.Sigmoid,
                     psname=f"r{name}")
                mm64(zt, [gw_sb[:, 2, :], gw_sb[:, 3, :]], [xgT, hT], AF.Sigmoid,
                     bias=negbz[:], psname=f"z{name}")
                for c0, cn in CH:
                    cs = slice(c0, c0 + cn)
                    warm(2)
                    nc.vector.tensor_mul(xr[:, cs], hT[:, cs], rt[:, cs])
                mm64(ht, [gw_sb[:, 4, :], gw_sb[:, 5, :]], [xgT, xr], AF.Tanh,
                     psname=f"h{name}")
                for c0, cn in CH:
                    cs = slice(c0, c0 + cn)
                    dst = (out_ap if out_ap is not None else out[:, :])
                    nc.vector.tensor_sub(ht[:, cs], ht[:, cs], hT[:, cs])
                    nc.vector.tensor_mul(ht[:, cs], ht[:, cs], zt[:, cs])
                    nc.vector.tensor_add(dst[:, cs], ht[:, cs], hT[:, cs])
            return out

        x1T = gru(xT, yT, g1w_sb, negbz1, "1")
        n2T = layer_norm(x1T[:, :], NTOK, CH, g2_row, b2_row, ng2_row[:], pbig, "2")
        eT = pbig.tile([D, NTOK], bf, tag="eT")
        with tc.tile_pool(name="pmlp", bufs=1) as pmlp:
            mT = pmlp.tile([MLP_D, NTOK], bf, tag="mT")
            mm64(mT, [we1_sb], [n2T], AF.Relu, psname="e1")
            mm64(eT, [we2_sb], [mT], AF.Relu, psname="e2")
        x2aug = pbig.tile([D + 1, NTOK], bf, tag="x2aug")
        nc.vector.memset(x2aug[D:D + 1, :], 1.0)
        gru(x1T, eT, g2w_sb, negbz2, "2", out_ap=x2aug[0:D, :])
        x2T = x2aug

        # ---------- voxel_mean out ----------
        MCH = [(128 * i, 128) for i in range(6)] + [(768, 32)]
        with tc.tile_pool(name="pvst", bufs=2) as pvst, \
             tc.tile_pool(name="ps_vo", bufs=4, space="PSUM") as ps_vop:
            VCH = [(512 * i, 512) for i in range(7)] + [(3584, 416)]
            HALF = [(0, 4, 2048), (4, 8, 1952)]
            for mi, (m0, mn) in enumerate(MCH):
                for hi, (na, nb, hw) in enumerate(HALF):
                    stg = pvst.tile([128, hw], f32, tag=f"vstg{hi}")
                    hbase = VCH[na][0]
                    for ni in range(na, nb):
                        v0, vn = VCH[ni]
                        ps = ps_vop.tile([128, 512], f32)
                        nc.tensor.matmul(ps[0:mn, 0:vn], x2aug[:, m0:m0 + mn],
                                         wvoxb[:, v0:v0 + vn],
                                         start=True, stop=True)
                        if ni % 2 == 0:
                            nc.scalar.copy(stg[0:mn, v0 - hbase:v0 - hbase + vn],
                                           ps[0:mn, 0:vn])
                        else:
                            nc.vector.tensor_copy(
                                stg[0:mn, v0 - hbase:v0 - hbase + vn],
                                ps[0:mn, 0:vn])
                    dma(out=ovox[m0:m0 + mn, hbase:hbase + hw], in_=stg[0:mn, :])

        # ---------- heads ----------
        with tc.tile_pool(name="ps_hd", bufs=1, space="PSUM") as ps_hd, \
             tc.tile_pool(name="phd", bufs=1) as phd:
            selb_ps = ps_hd.tile([D, 2, 512], f32, tag="selb")
            seled = phd.tile([D, NTOK], bf, tag="seled")
            for ci, (c0, cn) in enumerate(CH):
                nc.tensor.matmul(selb_ps[:, ci, 0:cn], ones1x64[:],
                                 sel_sb[:, c0:c0 + cn], start=True, stop=True)
                nc.vector.tensor_mul(seled[:, c0:c0 + cn], x2T[0:D, c0:c0 + cn],
                                     selb_ps[:, ci, 0:cn])
            lastT = phd.tile([D, BC], f32, tag="lastT")
            nc.vector.tensor_reduce(lastT[:],
                                    seled[:].rearrange("d (b t) -> d b t", b=BC),
                                    axis=AX.X, op=OP.add)
            lastb = phd.tile([D, BC], bf, tag="lastb")
            nc.vector.tensor_copy(lastb[:], lastT[:])
            ps_v1 = ps_hd.tile([D, BC], f32, tag="v1")
            nc.tensor.matmul(ps_v1[:], wval1_sb, lastb[:], start=True, stop=True)
            v1 = phd.tile([D, BC], bf, tag="v1s")
            nc.scalar.activation(v1[:], ps_v1[:], AF.Relu, bias=bval1_c)
            ps_vv = ps_hd.tile([1, BC], f32, tag="vv")
            nc.tensor.matmul(ps_vv[:], wval2_sb, v1[:], start=True, stop=True)
            valo = phd.tile([1, BC], f32, tag="valo")
            nc.scalar.activation(valo[:], ps_vv[:], AF.Identity, bias=bval2_c)
            dma(out=oval[:], in_=valo[:])
            ps_a1 = ps_hd.tile([D, BC], f32, tag="a1")
            nc.tensor.matmul(ps_a1[:], wa1_sb, lastb[:], start=True, stop=True)
            a1 = phd.tile([D, BC], bf, tag="a1s")
            nc.scalar.activation(a1[:], ps_a1[:], AF.Relu, bias=ba1_c)
            ps_ao = ps_hd.tile([NOUT, BC], f32, tag="aco")
            nc.tensor.matmul(ps_ao[:], wa2_sb, a1[:], start=True, stop=True)
            acto = phd.tile([NOUT, BC], f32, tag="acto")
            nc.scalar.activation(acto[:], ps_ao[:], AF.Identity, bias=ba2_c)
            dma(out=oact[:], in_=acto[:])

    nc.compile()
    return nc


def _prep_inputs(gaussians, gaussian_num, all_past_voxels, state0, params):
    gaussians = np.asarray(gaussians, np.float32)
    lengths = np.asarray(gaussian_num).astype(np.int64)
    vox_full = np.asarray(all_past_voxels)
    state0 = np.asarray(state0, np.float32)
    P = {k: (np.asarray(v, np.float32) if not isinstance(v, dict) else
             {k2: np.asarray(v2, np.float32) for k2, v2 in v.items()})
         for k, v in params.items()}
    minlen = int(lengths.min())
    ii = np.arange(T)

    b64 = np.zeros((D, NB64), np.float32)

    def put(name, arr):
        o, w = _B64[name]
        arr = np.asarray(arr, np.float32)
        if arr.ndim == 1:
            arr = arr[:, None]
        b64[:arr.shape[0], o:o + w] = arr

    put("wqkv", P["w_qkv"]); put("wpos", P["w_pos"]); put("wattn", P["w_attn"])
    put("g1w", np.concatenate([P["gru1"][k] for k in
                               ("wr", "ur", "wz", "uz", "wh", "uh")], axis=1))
    put("g2w", np.concatenate([P["gru2"][k] for k in
                               ("wr", "ur", "wz", "uz", "wh", "uh")], axis=1))
    put("we1", P["w_e1"]); put("we2", P["w_e2"])
    put("wa1", P["w_a1"]); put("wa2", P["w_a2"])
    put("wval1", P["w_val1"]); put("wval2", P["w_val2"])
    put("relposT", _relpos_table())
    put("ba1", P["b_a1"]); put("bval1", P["b_val1"])
    put("ba2", P["b_a2"]); put("bval2", P["b_val2"].reshape(1))
    put("g1bz", P["gru1"]["bz"]); put("g2bz", P["gru2"]["bz"])
    put("u", P["uvar"].reshape(D)); put("v", P["vvar"].reshape(D))
    put("wg", np.concatenate([P["w_g"], P["b_g"][None, :]], 0))

    blobr = np.zeros((1, NBR), np.float32)
    blobr[0, 0:64] = P["ln1_g"]; blobr[0, 64:128] = P["ln1_b"]
    blobr[0, 128:192] = P["ln2_g"]; blobr[0, 192:256] = P["ln2_b"]
    blobr[0, 256:304] = P["b_v"]

    causal = (np.arange(FT)[None, :] < (TAU + 1 + ii)[:, None]).astype(np.float32)
    wvm = np.ascontiguousarray(
        P["w_v"].reshape(NCELL, NMAT, VFEAT).transpose(1, 0, 2), np.float32)

    in_maps = []
    for c in range(NCORES):
        sl = slice(c * BC, (c + 1) * BC)
        g = gaussians[sl, :T].reshape(BC * T, GF).T
        gaug = np.concatenate([g, np.ones((1, NTOK), np.float32)], 0)
        v = vox_full[sl, :T].reshape(BC, T, NCELL).copy()
        v[:, 0] = 0
        voxT = np.ascontiguousarray(v.reshape(NTOK, NCELL).T.astype(np.int8))
        st = np.ascontiguousarray(state0[sl].reshape(BC * TAU, D).T)
        c0r = np.ones(NTOK, np.float32)
        c0r[0::T] = 0.0
        a2 = np.ascontiguousarray(np.stack([np.ones(NTOK, np.float32), c0r]))
        ln = lengths[sl]
        valid = (ii[:, None] <= ln[None, :]).astype(np.float32)       # [100, 8]
        avm = valid * (ii[:, None] < TAU + 1 + minlen).astype(np.float32)
        selr = np.zeros((1, NTOK), np.float32)
        for b in range(BC):
            selr[0, b * T + int(ln[b])] = 1.0
        b100 = np.zeros((T, NB100), np.float32)
        b100[:, 0:BC] = valid
        b100[:, BC:2 * BC] = avm
        b100[:, 2 * BC:2 * BC + FT] = causal

        m = {"vox": voxT, "gaus": np.ascontiguousarray(gaug).astype(BF16),
             "st0": st.astype(BF16), "aug2": a2.astype(BF16),
             "sel": selr.astype(BF16),
             "blob64": b64.astype(BF16), "blob100": b100, "blobr": blobr,
             "wvm": wvm.astype(BF16), "wvox": P["w_vox"].astype(BF16),
             "bvox": P["b_vox"].astype(BF16),
             "bvr": P["b_v"].reshape(1, VFEAT).astype(BF16)}
        in_maps.append(m)
    return in_maps


def kernel(gaussians, gaussian_num, all_past_voxels, state0, params):
    from concourse.bass_utils import run_bass_kernel_spmd
    if "nc" not in _CACHE:
        _CACHE["nc"] = _build()
    nc = _CACHE["nc"]
    in_maps = _prep_inputs(gaussians, gaussian_num, all_past_voxels, state0, params)
    res = run_bass_kernel_spmd(nc, in_maps, core_ids=list(range(NCORES)))
    acts, voxs, vals = [], [], []
    for c in range(NCORES):
        r = res.results[c]
        acts.append(np.ascontiguousarray(r["oact"].T))
        voxs.append(r["ovox"].reshape(BC, T, VDIM))
        vals.append(r["oval"].reshape(BC))
    act = np.concatenate(acts, 0).astype(np.float32)
    voxm = np.concatenate(voxs, 0).astype(np.float32)
    val = np.concatenate(vals, 0).astype(np.float32)
    return act, voxm, val
